# revision 51
# baseline (speedup 1.0000x reference)
"""Trainium2 Bass kernel for nn_GrassmannNN.

Math: the reference's Z2-graded (Grassmann) network collapses per-sample to a
chain of 32x32 matmuls selected by the sample's bits.  For each body layer m
(embedding e0/e1, core G = body_w[m]):
  bit=0:  x <- tanh(x @ M0_m),  M0_m = (sum_{j<16} e0_j G[:,j,:]) * blockdiag
  bit=1:  x <- tanh(x @ M1_m),  M1_m = (sum_{j>=16} e1_j G[:,j,:]) * antidiag/sign
Head: x0 built from embedding[0] by bit0, then tanh(x0 @ (head_w * blockdiag)).

Only 256 distinct bit patterns exist.  The pattern table is built by PREFIX
DOUBLING: S_1 (32,2) -> ... -> S_7 (32,128), where
  S_{i+1}[:, 0:2^i]       = M0_i.T @ S_i        (bit_i = 0)
  S_{i+1}[:, 2^i:2^{i+1}] = M1_i.T @ S_i        (bit_i = 1)
so total table matmul columns are 2+4+...+256 instead of 8*256.  The final
step is emitted transposed (patterns on partitions) giving the (256, 64)
output table directly.  Transition matrices for all 7 layers come from ONE
fused pair of fp32 matmuls (K = (layer, j) blocks of 128/96) followed by a
masked psum->SBUF copy and a reshaping SBUF->SBUF DMA.  The network is
chaotic (~1e4 error amplification through the 8 tanh layers), so the whole
table build stays fp32; only the final table values are cast to bf16 (a
last-stage, unamplified 0.4% rounding) so the one-hot gather matmuls run in
1-pass bf16.  A short burst of dummy bf16 matmuls at kernel start warms the
PE HAM clock gate (1.2 -> 2.4 GHz) while input DMAs stream.

Sharding: pure data parallel over the batch; each of the 8 cores computes the
(tiny) table redundantly and gathers its 1024-sample slice.
"""

import numpy as np
from contextlib import ExitStack

import concourse.bass as bass
import concourse.bacc as bacc
import concourse.tile as tile
import concourse.mybir as mybir
from concourse.bass_utils import run_bass_kernel_spmd

F32 = mybir.dt.float32
BF16 = mybir.dt.bfloat16
AF = mybir.ActivationFunctionType
OP = mybir.AluOpType

NCORES = 8
B = 8192
BC = B // NCORES          # 1024 samples per core
N = 256                   # distinct bit patterns
NWARM = 7                 # dummy matmuls to warm the PE clock gate


def _host_consts():
    pd = np.concatenate([np.zeros(16, np.int64), np.ones(16, np.int64)])
    maskbd = ((pd[:, None] ^ pd[None, :]) == 0).astype(np.float32)
    maskads = (((pd[:, None] ^ pd[None, :]) == 1).astype(np.float32)
               * (1.0 - 2.0 * pd)[:, None].astype(np.float32))
    blob = np.zeros((128, 50), np.float32)
    # emask0 cols 0:8 : rows p=m*32+j (m=0..3 -> sites 1..4), col m*2 + (j>=16)
    for m in range(4):
        for j in range(32):
            blob[m * 32 + j, m * 2 + (j // 16)] = 1.0
    # emask1 cols 8:14 : rows p=m*32+j (m=0..2 -> sites 5..7)
    for m in range(3):
        for j in range(32):
            blob[m * 32 + j, 8 + m * 2 + (j // 16)] = 1.0
    blob[0:16, 14] = 1.0          # elh2 col 0
    blob[16:32, 15] = 1.0         # elh2 col 1
    blob[0:32, 16:48] = maskbd
    blob[0:128, 48] = np.arange(128, dtype=np.float32)
    blob[0:128, 49] = np.arange(128, 256, dtype=np.float32)
    pow2 = np.broadcast_to((2.0 ** np.arange(8, dtype=np.float32))[:, None],
                           (8, 128))
    mrep2 = np.tile(np.concatenate([maskbd, maskads], 1), (1, 7))  # (32, 448)
    return blob, np.ascontiguousarray(mrep2, dtype=np.float32), pow2


def _emit(ctx: ExitStack, tc, t):
    nc = tc.nc
    cpool = ctx.enter_context(tc.tile_pool(name="consts", bufs=1))
    mpool = ctx.enter_context(tc.tile_pool(name="mats", bufs=1))
    spool = ctx.enter_context(tc.tile_pool(name="state", bufs=2))
    psE = ctx.enter_context(tc.tile_pool(name="psE", bufs=1, space="PSUM"))
    psS = ctx.enter_context(tc.tile_pool(name="psS", bufs=2, space="PSUM"))
    psG = ctx.enter_context(tc.tile_pool(name="psG", bufs=1, space="PSUM"))

    # ---- PE warm-up: dummy bf16 matmuls (into pe0, overwritten later) ----
    pe0 = psE.tile([8, 1024], F32, tag="pe0")
    wtile = cpool.tile([128, 512], BF16)
    nc.vector.memset(wtile[:], 0.0)
    for _ in range(NWARM):
        nc.tensor.matmul(pe0[:, 0:512], wtile[:, 0:8], wtile[:],
                         start=True, stop=True)

    # ---- input DMAs (balls split for earlier first-half completion) ----
    tBall0 = cpool.tile([128, 1024], F32)
    nc.sync.dma_start(tBall0[:, 0:512], t["ball0"].ap()[:, 0:512])
    nc.sync.dma_start(tBall0[:, 512:1024], t["ball0"].ap()[:, 512:1024])
    tMrep2 = cpool.tile([32, 448], F32)
    nc.sync.dma_start(tMrep2[:], t["mrep2"].ap())
    tBlob = cpool.tile([128, 50], F32)
    nc.scalar.dma_start(tBlob[:], t["blob"].ap())
    tEv0 = cpool.tile([128, 1], F32)
    nc.scalar.dma_start(tEv0[:], bass.AP(t["emb"], 32, [[1, 128], [1, 1]]))
    tEv1 = cpool.tile([96, 1], F32)
    nc.scalar.dma_start(tEv1[:], bass.AP(t["emb"], 160, [[1, 96], [1, 1]]))
    tEvH = cpool.tile([32, 1], F32)
    nc.scalar.dma_start(tEvH[:], bass.AP(t["emb"], 0, [[1, 32], [1, 1]]))
    tBall1 = cpool.tile([96, 1024], F32)
    nc.scalar.dma_start(tBall1[:], t["ball1"].ap())
    tDat = cpool.tile([8, BC], BF16)
    nc.scalar.dma_start(tDat[:], t["dataT"].ap())
    tHead = cpool.tile([32, 32], F32)
    nc.scalar.dma_start(tHead[:], t["head"].ap())
    tPow = mpool.tile([8, 128], BF16)
    nc.gpsimd.dma_start(tPow[:], t["pow2"].ap())

    # ---- small operand builds (DVE) ----
    tEbd0 = mpool.tile([128, 8], F32)
    nc.vector.tensor_scalar(tEbd0[:], tBlob[:, 0:8], tEv0[:], None, OP.mult)
    tEbd1 = mpool.tile([96, 6], F32)
    nc.vector.tensor_scalar(tEbd1[:], tBlob[0:96, 8:14],
                            tEv1[:], None, OP.mult)
    tMh = mpool.tile([32, 32], F32)
    nc.vector.tensor_mul(tMh[:], tHead[:], tBlob[0:32, 16:48])
    tX0 = mpool.tile([32, 2], F32)
    nc.vector.tensor_scalar(tX0[:], tBlob[0:32, 14:16],
                            tEvH[:], None, OP.mult)

    # ---- fused E-contraction: all 7 layers' (M0, M1) in two K-blocks ----
    # psum -> SBUF copy on the (idle) ACT engine, then one single-partition
    # SBUF->SBUF DMA per M matrix (round-robin over 4 queues), then the
    # mask applied per layer-pair on the (idle) GpSimd engine.
    tMallR = mpool.tile([32, 448], F32)
    tMall = mpool.tile([32, 448], F32)

    nc.tensor.matmul(pe0[:, 0:512], tEbd0[:], tBall0[:, 0:512],
                     start=True, stop=True)
    nc.tensor.matmul(pe0[:, 512:1024], tEbd0[:], tBall0[:, 512:1024],
                     start=True, stop=True)
    # psum -> SBUF copies as single ACT ops (ACT is idle; one op per block
    # avoids the scheduler re-ordering a second half behind other work)
    tWr0 = mpool.tile([8, 1024], F32)
    nc.scalar.activation(tWr0[:], pe0[:], AF.Copy)

    # head emitted between the two E-contract blocks: its tiny matmul slots
    # into the E0->E1 boundary so S1's tanh retires early -- a late S1 ahead
    # of the reshape DMAs in Tile's predicted scalar order inflates the
    # cross-engine wait ticks of everything queued behind it
    ps1 = psS.tile([32, 2], F32, tag="ps")
    nc.tensor.matmul(ps1[:], tMh[:], tX0[:], start=True, stop=True)
    S = spool.tile([32, 2], F32, tag="S1")
    nc.scalar.activation(S[:], ps1[:], AF.Tanh)

    pe1 = psE.tile([6, 1024], F32, tag="pe1")
    nc.tensor.matmul(pe1[:, 0:512], tEbd1[:], tBall1[:, 0:512],
                     start=True, stop=True)
    nc.tensor.matmul(pe1[:, 512:1024], tEbd1[:], tBall1[:, 512:1024],
                     start=True, stop=True)
    tWr1 = mpool.tile([6, 1024], F32)
    nc.scalar.activation(tWr1[:], pe1[:], AF.Copy)

    def mslice2(m):
        off = m * 64 if m < 4 else 256 + (m - 4) * 64
        return tMall[:, off:off + 64]

    for m in range(7):
        off = m * 64 if m < 4 else 256 + (m - 4) * 64
        src = tWr0 if m < 4 else tWr1
        lb0 = (m * 2) if m < 4 else ((m - 4) * 2)
        for b in range(2):
            # (1, 1024) single-partition row -> (32, 32) reshape DMA
            q = nc.sync if b == 0 else nc.gpsimd
            q.dma_start(tMallR[:, off + b * 32:off + b * 32 + 32],
                        src[lb0 + b:lb0 + b + 1, :])
        nc.vector.tensor_mul(tMall[:, off:off + 64], tMallR[:, off:off + 64],
                             tMrep2[:, off:off + 64])

    # ---- prefix-doubling rollout: S_i (32, 2^i) -> S_7 (32, 128) ----
    # one matmul per step with stacked lhsT [M0 | M1] -> psum (64, 2^i);
    # the sample-index one-hot path is interleaved so its PE matmuls fill
    # rollout gaps and its DVE is_equal ops land after the mask mults
    tOh0 = cpool.tile([128, BC], BF16)
    tOh1 = cpool.tile([128, BC], BF16)

    def idxpath(h):
        # the index matmuls share the pgA psum slot (disjoint lifetime)
        pid = psG.tile([128, 512], F32, tag="pgA")
        nc.tensor.matmul(pid[:], tPow[:], tDat[:, h * 512:(h + 1) * 512],
                         start=True, stop=True)
        nc.vector.tensor_scalar(tOh0[:, h * 512:(h + 1) * 512], pid[:],
                                tBlob[:, 48:49], None, OP.is_equal)
        nc.vector.tensor_scalar(tOh1[:, h * 512:(h + 1) * 512], pid[:],
                                tBlob[:, 49:50], None, OP.is_equal)

    for i in range(1, 7):
        w = 1 << i
        ps = psS.tile([64, 64], F32, tag="ps")
        nc.tensor.matmul(ps[:, 0:w], mslice2(i - 1), S[:],
                         start=True, stop=True)
        S2 = spool.tile([32, 2 * w], F32, tag=f"S{i + 1}")
        nc.scalar.activation(S2[:, 0:w], ps[0:32, 0:w], AF.Tanh)
        nc.scalar.activation(S2[:, w:2 * w], ps[32:64, 0:w], AF.Tanh)
        S = S2
        if i == 3:
            idxpath(0)
        elif i == 4:
            idxpath(1)

    # ---- final layer transposed: bf16 table halves (128 patterns, 64) ----
    pf = psG.tile([128, 512], F32, tag="pgB")
    nc.tensor.matmul(pf[:, 0:64], S[:], mslice2(6), start=True, stop=True)
    Ths = []
    for b in range(2):
        T = mpool.tile([128, 64], BF16, tag=f"T{b}")
        nc.gpsimd.memset(T[:], 0.0)
        # one ACT per half: out cols {0:16} u {48:64} via strided 3D APs
        tout = T[:].rearrange("p (a c) -> p a c", a=4, c=16)[:, ::3, :]
        tin = pf[:].rearrange("p (a c) -> p a c", a=32, c=16)[:, 2 * b:2 * b + 2, :]
        nc.scalar.activation(tout, tin, AF.Tanh)
        Ths.append(T)

    # ---- gather: one-hot bf16 matmuls, two psum banks for early writeback ----
    pgA = psG.tile([128, 256], F32, tag="pgA")
    pgB = psG.tile([128, 256], F32, tag="pgB")
    pgs = [pgA, pgB]
    # (pgA reuses the index-matmul slot, pgB the final-layer slot)
    for bt in range(8):
        pgx = pgs[bt // 4]
        c0 = (bt % 4) * 64
        nc.tensor.matmul(pgx[:, c0:c0 + 64],
                         tOh0[:, bt * 128:(bt + 1) * 128], Ths[0][:],
                         start=True, stop=False)
        nc.tensor.matmul(pgx[:, c0:c0 + 64],
                         tOh1[:, bt * 128:(bt + 1) * 128], Ths[1][:],
                         start=False, stop=True)

    # ---- stage (DVE + ACT in parallel) + write out (both HWDGE queues) ----
    og0 = mpool.tile([128, 256], F32, tag="og0")
    nc.vector.tensor_copy(og0[:], pgs[0][:])
    nc.sync.dma_start(bass.AP(t["out"], 0, [[64, 128], [8192, 4], [1, 64]]),
                      og0[:].rearrange("p (b c) -> p b c", b=4))
    og1 = mpool.tile([128, 256], F32, tag="og1")
    nc.scalar.activation(og1[:], pgs[1][:], AF.Copy)
    nc.scalar.dma_start(bass.AP(t["out"], 32768, [[64, 128], [8192, 4], [1, 64]]),
                        og1[:].rearrange("p (b c) -> p b c", b=4))


def build_program():
    nc = bacc.Bacc("TRN2", target_bir_lowering=False, debug=False,
                   enable_asserts=False, num_devices=NCORES)
    t = {}
    t["ball0"] = nc.dram_tensor("ball0", [128, 1024], F32, kind="ExternalInput")
    t["ball1"] = nc.dram_tensor("ball1", [96, 1024], F32, kind="ExternalInput")
    t["blob"] = nc.dram_tensor("blob", [128, 50], F32, kind="ExternalInput")
    t["mrep2"] = nc.dram_tensor("mrep2", [32, 448], F32, kind="ExternalInput")
    t["pow2"] = nc.dram_tensor("pow2", [8, 128], BF16, kind="ExternalInput")
    t["dataT"] = nc.dram_tensor("dataT", [8, BC], BF16, kind="ExternalInput")
    t["emb"] = nc.dram_tensor("emb", [256], F32, kind="ExternalInput")
    t["head"] = nc.dram_tensor("head", [32, 32], F32, kind="ExternalInput")
    t["out"] = nc.dram_tensor("out", [BC, 64], F32, kind="ExternalOutput")
    with tile.TileContext(nc) as tc:
        with ExitStack() as ctx:
            _emit(ctx, tc, t)
    nc.compile()
    return nc


def make_in_maps(data, embedding, head_w, body_w):
    import ml_dtypes
    bf = ml_dtypes.bfloat16
    data = np.asarray(data)
    if data.dtype == np.int64:
        d32 = data.view(np.int32).reshape(B, 16)[:, ::2]
    else:
        d32 = data.astype(np.int32, copy=False)
    blob, mrep2, pow2 = _host_consts()
    ballf = np.ascontiguousarray(
        np.asarray(body_w, np.float32).transpose(0, 2, 1, 3)).reshape(224, 1024)
    base = {
        "ball0": np.ascontiguousarray(ballf[0:128]),
        "ball1": np.ascontiguousarray(ballf[128:224]),
        "blob": blob,
        "mrep2": mrep2,
        "pow2": np.ascontiguousarray(pow2.astype(np.float32)).astype(bf),
        "emb": np.ascontiguousarray(embedding, np.float32).reshape(-1),
        "head": np.ascontiguousarray(head_w, np.float32),
    }
    in_maps = []
    for c in range(NCORES):
        dslice = np.ascontiguousarray(
            d32[c * BC:(c + 1) * BC].T).astype(np.float32).astype(bf)
        in_maps.append({**base, "dataT": dslice})
    return in_maps


_CACHE = {}


def kernel(data, embedding, head_w, body_w, **kw):
    nc = _CACHE.get("nc")
    if nc is None:
        nc = build_program()
        _CACHE["nc"] = nc
    in_maps = make_in_maps(data, embedding, head_w, body_w)
    res = run_bass_kernel_spmd(nc, in_maps, core_ids=list(range(NCORES)))
    out = np.concatenate([res.results[c]["out"] for c in range(NCORES)], axis=0)
    return out.reshape(B, 2, 32)


# revision 52
# speedup vs baseline: 1.0843x; 1.0843x over previous
"""Trainium2 Bass kernel for nn_GrassmannNN.

Math: the reference's Z2-graded (Grassmann) network collapses per-sample to a
chain of 32x32 matmuls selected by the sample's bits.  For each body layer m
(embedding e0/e1, core G = body_w[m]):
  bit=0:  x <- tanh(x @ M0_m),  M0_m = (sum_{j<16} e0_j G[:,j,:]) * blockdiag
  bit=1:  x <- tanh(x @ M1_m),  M1_m = (sum_{j>=16} e1_j G[:,j,:]) * antidiag/sign
Head: x0 built from embedding[0] by bit0, then tanh(x0 @ (head_w * blockdiag)).

Only 256 distinct bit patterns exist.  The pattern table is built by PREFIX
DOUBLING: S_1 (32,2) -> ... -> S_7 (32,128), where
  S_{i+1}[:, 0:2^i]       = M0_i.T @ S_i        (bit_i = 0)
  S_{i+1}[:, 2^i:2^{i+1}] = M1_i.T @ S_i        (bit_i = 1)
so total table matmul columns are 2+4+...+256 instead of 8*256.  The final
step is emitted transposed (patterns on partitions) giving the (256, 64)
output table directly.  Transition matrices for all 7 layers come from ONE
fused pair of fp32 matmuls (K = (layer, j) blocks of 128/96) followed by a
masked psum->SBUF copy and a reshaping SBUF->SBUF DMA.  The network is
chaotic (~1e4 error amplification through the 8 tanh layers), so the whole
table build stays fp32; only the final table values are cast to bf16 (a
last-stage, unamplified 0.4% rounding) so the one-hot gather matmuls run in
1-pass bf16.  A short burst of dummy bf16 matmuls at kernel start warms the
PE HAM clock gate (1.2 -> 2.4 GHz) while input DMAs stream.

Sharding: pure data parallel over the batch; each of the 8 cores computes the
(tiny) table redundantly and gathers its 1024-sample slice.
"""

import numpy as np
from contextlib import ExitStack

import concourse.bass as bass
import concourse.bacc as bacc
import concourse.tile as tile
import concourse.mybir as mybir
from concourse.bass_utils import run_bass_kernel_spmd

F32 = mybir.dt.float32
BF16 = mybir.dt.bfloat16
AF = mybir.ActivationFunctionType
OP = mybir.AluOpType

NCORES = 8
B = 8192
BC = B // NCORES          # 1024 samples per core
N = 256                   # distinct bit patterns
NWARM = 7                 # dummy matmuls to warm the PE clock gate


def _host_consts():
    pd = np.concatenate([np.zeros(16, np.int64), np.ones(16, np.int64)])
    maskbd = ((pd[:, None] ^ pd[None, :]) == 0).astype(np.float32)
    maskads = (((pd[:, None] ^ pd[None, :]) == 1).astype(np.float32)
               * (1.0 - 2.0 * pd)[:, None].astype(np.float32))
    blob = np.zeros((128, 50), np.float32)
    # emask0 cols 0:8 : rows p=m*32+j (m=0..3 -> sites 1..4), col m*2 + (j>=16)
    for m in range(4):
        for j in range(32):
            blob[m * 32 + j, m * 2 + (j // 16)] = 1.0
    # emask1 cols 8:14 : rows p=m*32+j (m=0..2 -> sites 5..7)
    for m in range(3):
        for j in range(32):
            blob[m * 32 + j, 8 + m * 2 + (j // 16)] = 1.0
    blob[0:16, 14] = 1.0          # elh2 col 0
    blob[16:32, 15] = 1.0         # elh2 col 1
    blob[0:32, 16:48] = maskbd
    blob[0:128, 48] = np.arange(128, dtype=np.float32)
    blob[0:128, 49] = np.arange(128, 256, dtype=np.float32)
    pow2 = np.broadcast_to((2.0 ** np.arange(8, dtype=np.float32))[:, None],
                           (8, 128))
    mrep2 = np.tile(np.concatenate([maskbd, maskads], 1), (1, 7))  # (32, 448)
    return blob, np.ascontiguousarray(mrep2, dtype=np.float32), pow2


def _emit(ctx: ExitStack, tc, t):
    nc = tc.nc
    cpool = ctx.enter_context(tc.tile_pool(name="consts", bufs=1))
    mpool = ctx.enter_context(tc.tile_pool(name="mats", bufs=1))
    spool = ctx.enter_context(tc.tile_pool(name="state", bufs=2))
    psE = ctx.enter_context(tc.tile_pool(name="psE", bufs=1, space="PSUM"))
    psS = ctx.enter_context(tc.tile_pool(name="psS", bufs=2, space="PSUM"))
    psG = ctx.enter_context(tc.tile_pool(name="psG", bufs=1, space="PSUM"))

    # ---- PE warm-up: dummy bf16 matmuls (into pe0, overwritten later) ----
    pe0 = psE.tile([8, 1024], F32, tag="pe0")
    wtile = cpool.tile([128, 512], BF16)
    nc.vector.memset(wtile[:], 0.0)
    for _ in range(NWARM):
        nc.tensor.matmul(pe0[:, 0:512], wtile[:, 0:8], wtile[:],
                         start=True, stop=True)

    # ---- input DMAs (balls split for earlier first-half completion) ----
    tBall0 = cpool.tile([128, 1024], F32)
    nc.sync.dma_start(tBall0[:, 0:512], t["ball0"].ap()[:, 0:512])
    nc.sync.dma_start(tBall0[:, 512:1024], t["ball0"].ap()[:, 512:1024])
    tMrep2 = cpool.tile([32, 448], F32)
    nc.sync.dma_start(tMrep2[:], t["mrep2"].ap())
    tBlob = cpool.tile([128, 50], F32)
    nc.scalar.dma_start(tBlob[:], t["blob"].ap())
    tEv0 = cpool.tile([128, 1], F32)
    nc.scalar.dma_start(tEv0[:], bass.AP(t["emb"], 32, [[1, 128], [1, 1]]))
    tEv1 = cpool.tile([96, 1], F32)
    nc.scalar.dma_start(tEv1[:], bass.AP(t["emb"], 160, [[1, 96], [1, 1]]))
    tEvH = cpool.tile([32, 1], F32)
    nc.scalar.dma_start(tEvH[:], bass.AP(t["emb"], 0, [[1, 32], [1, 1]]))
    tBall1 = cpool.tile([96, 1024], F32)
    nc.scalar.dma_start(tBall1[:], t["ball1"].ap())
    tDat = cpool.tile([8, BC], BF16)
    nc.scalar.dma_start(tDat[:], t["dataT"].ap())
    tHead = cpool.tile([32, 32], F32)
    nc.scalar.dma_start(tHead[:], t["head"].ap())
    tPow = mpool.tile([8, 128], BF16)
    nc.gpsimd.dma_start(tPow[:], t["pow2"].ap())

    # ---- small operand builds (DVE) ----
    tEbd0 = mpool.tile([128, 8], F32)
    nc.vector.tensor_scalar(tEbd0[:], tBlob[:, 0:8], tEv0[:], None, OP.mult)
    tEbd1 = mpool.tile([96, 6], F32)
    nc.vector.tensor_scalar(tEbd1[:], tBlob[0:96, 8:14],
                            tEv1[:], None, OP.mult)
    tMh = mpool.tile([32, 32], F32)
    nc.vector.tensor_mul(tMh[:], tHead[:], tBlob[0:32, 16:48])
    tX0 = mpool.tile([32, 2], F32)
    nc.vector.tensor_scalar(tX0[:], tBlob[0:32, 14:16],
                            tEvH[:], None, OP.mult)

    # ---- fused E-contraction: all 7 layers' (M0, M1) in two K-blocks ----
    # psum -> SBUF copy on the (idle) ACT engine, then one single-partition
    # SBUF->SBUF DMA per M matrix (round-robin over 4 queues), then the
    # mask applied per layer-pair on the (idle) GpSimd engine.
    tMallR = mpool.tile([32, 448], F32)
    tMall = mpool.tile([32, 448], F32)

    nc.tensor.matmul(pe0[:, 0:512], tEbd0[:], tBall0[:, 0:512],
                     start=True, stop=True)
    nc.tensor.matmul(pe0[:, 512:1024], tEbd0[:], tBall0[:, 512:1024],
                     start=True, stop=True)
    # psum -> SBUF copies as single ACT ops (ACT is idle; one op per block
    # avoids the scheduler re-ordering a second half behind other work)
    tWr0 = mpool.tile([8, 1024], F32)
    nc.scalar.activation(tWr0[:], pe0[:], AF.Copy)

    # head emitted between the two E-contract blocks: its tiny matmul slots
    # into the E0->E1 boundary so S1's tanh retires early -- a late S1 ahead
    # of the reshape DMAs in Tile's predicted scalar order inflates the
    # cross-engine wait ticks of everything queued behind it
    ps1 = psS.tile([32, 2], F32, tag="ps")
    nc.tensor.matmul(ps1[:], tMh[:], tX0[:], start=True, stop=True)
    S = spool.tile([32, 2], F32, tag="S1")
    nc.scalar.activation(S[:], ps1[:], AF.Tanh)

    pe1 = psE.tile([6, 1024], F32, tag="pe1")
    nc.tensor.matmul(pe1[:, 0:512], tEbd1[:], tBall1[:, 0:512],
                     start=True, stop=True)
    nc.tensor.matmul(pe1[:, 512:1024], tEbd1[:], tBall1[:, 512:1024],
                     start=True, stop=True)
    tWr1 = mpool.tile([6, 1024], F32)
    nc.scalar.activation(tWr1[:], pe1[:], AF.Copy)

    def mslice2(m):
        off = m * 64 if m < 4 else 256 + (m - 4) * 64
        return tMall[:, off:off + 64]

    for m in range(7):
        off = m * 64 if m < 4 else 256 + (m - 4) * 64
        src = tWr0 if m < 4 else tWr1
        lb0 = (m * 2) if m < 4 else ((m - 4) * 2)
        for b in range(2):
            # (1, 1024) single-partition row -> (32, 32) reshape DMA; all on
            # the sync queue in pair order so the gpsimd queue holds ONLY the
            # masks (no DMA issues to head-of-line-block them)
            nc.sync.dma_start(tMallR[:, off + b * 32:off + b * 32 + 32],
                              src[lb0 + b:lb0 + b + 1, :])
        nc.gpsimd.tensor_mul(tMall[:, off:off + 64], tMallR[:, off:off + 64],
                             tMrep2[:, off:off + 64])

    # ---- prefix-doubling rollout: S_i (32, 2^i) -> S_7 (32, 128) ----
    # one matmul per step with stacked lhsT [M0 | M1] -> psum (64, 2^i);
    # the sample-index one-hot path is interleaved so its PE matmuls fill
    # rollout gaps and its DVE is_equal ops land after the mask mults
    tOh0 = cpool.tile([128, BC], BF16)
    tOh1 = cpool.tile([128, BC], BF16)

    def idxpath(h):
        # the index matmuls share the pgA psum slot (disjoint lifetime)
        pid = psG.tile([128, 512], F32, tag="pgA")
        nc.tensor.matmul(pid[:], tPow[:], tDat[:, h * 512:(h + 1) * 512],
                         start=True, stop=True)
        nc.vector.tensor_scalar(tOh0[:, h * 512:(h + 1) * 512], pid[:],
                                tBlob[:, 48:49], None, OP.is_equal)
        nc.vector.tensor_scalar(tOh1[:, h * 512:(h + 1) * 512], pid[:],
                                tBlob[:, 49:50], None, OP.is_equal)

    for i in range(1, 7):
        w = 1 << i
        ps = psS.tile([64, 64], F32, tag="ps")
        nc.tensor.matmul(ps[:, 0:w], mslice2(i - 1), S[:],
                         start=True, stop=True)
        S2 = spool.tile([32, 2 * w], F32, tag=f"S{i + 1}")
        nc.scalar.activation(S2[:, 0:w], ps[0:32, 0:w], AF.Tanh)
        nc.scalar.activation(S2[:, w:2 * w], ps[32:64, 0:w], AF.Tanh)
        S = S2
        if i == 3:
            idxpath(0)
        elif i == 4:
            idxpath(1)

    # ---- final layer transposed: bf16 table halves (128 patterns, 64) ----
    pf = psG.tile([128, 512], F32, tag="pgB")
    nc.tensor.matmul(pf[:, 0:64], S[:], mslice2(6), start=True, stop=True)
    Ths = []
    for b in range(2):
        T = mpool.tile([128, 64], BF16, tag=f"T{b}")
        nc.gpsimd.memset(T[:], 0.0)
        # one ACT per half: out cols {0:16} u {48:64} via strided 3D APs
        tout = T[:].rearrange("p (a c) -> p a c", a=4, c=16)[:, ::3, :]
        tin = pf[:].rearrange("p (a c) -> p a c", a=32, c=16)[:, 2 * b:2 * b + 2, :]
        nc.scalar.activation(tout, tin, AF.Tanh)
        Ths.append(T)

    # ---- gather: one-hot bf16 matmuls, two psum banks for early writeback ----
    pgA = psG.tile([128, 256], F32, tag="pgA")
    pgB = psG.tile([128, 256], F32, tag="pgB")
    pgs = [pgA, pgB]
    # (pgA reuses the index-matmul slot, pgB the final-layer slot)
    for bt in range(8):
        pgx = pgs[bt // 4]
        c0 = (bt % 4) * 64
        nc.tensor.matmul(pgx[:, c0:c0 + 64],
                         tOh0[:, bt * 128:(bt + 1) * 128], Ths[0][:],
                         start=True, stop=False)
        nc.tensor.matmul(pgx[:, c0:c0 + 64],
                         tOh1[:, bt * 128:(bt + 1) * 128], Ths[1][:],
                         start=False, stop=True)

    # ---- stage (DVE + ACT in parallel) + write out (both HWDGE queues) ----
    og0 = mpool.tile([128, 256], F32, tag="og0")
    nc.vector.tensor_copy(og0[:], pgs[0][:])
    nc.sync.dma_start(bass.AP(t["out"], 0, [[64, 128], [8192, 4], [1, 64]]),
                      og0[:].rearrange("p (b c) -> p b c", b=4))
    og1 = mpool.tile([128, 256], F32, tag="og1")
    nc.scalar.activation(og1[:], pgs[1][:], AF.Copy)
    nc.scalar.dma_start(bass.AP(t["out"], 32768, [[64, 128], [8192, 4], [1, 64]]),
                        og1[:].rearrange("p (b c) -> p b c", b=4))


def build_program():
    nc = bacc.Bacc("TRN2", target_bir_lowering=False, debug=False,
                   enable_asserts=False, num_devices=NCORES)
    t = {}
    t["ball0"] = nc.dram_tensor("ball0", [128, 1024], F32, kind="ExternalInput")
    t["ball1"] = nc.dram_tensor("ball1", [96, 1024], F32, kind="ExternalInput")
    t["blob"] = nc.dram_tensor("blob", [128, 50], F32, kind="ExternalInput")
    t["mrep2"] = nc.dram_tensor("mrep2", [32, 448], F32, kind="ExternalInput")
    t["pow2"] = nc.dram_tensor("pow2", [8, 128], BF16, kind="ExternalInput")
    t["dataT"] = nc.dram_tensor("dataT", [8, BC], BF16, kind="ExternalInput")
    t["emb"] = nc.dram_tensor("emb", [256], F32, kind="ExternalInput")
    t["head"] = nc.dram_tensor("head", [32, 32], F32, kind="ExternalInput")
    t["out"] = nc.dram_tensor("out", [BC, 64], F32, kind="ExternalOutput")
    with tile.TileContext(nc) as tc:
        with ExitStack() as ctx:
            _emit(ctx, tc, t)
    nc.compile()
    return nc


def make_in_maps(data, embedding, head_w, body_w):
    import ml_dtypes
    bf = ml_dtypes.bfloat16
    data = np.asarray(data)
    if data.dtype == np.int64:
        d32 = data.view(np.int32).reshape(B, 16)[:, ::2]
    else:
        d32 = data.astype(np.int32, copy=False)
    blob, mrep2, pow2 = _host_consts()
    ballf = np.ascontiguousarray(
        np.asarray(body_w, np.float32).transpose(0, 2, 1, 3)).reshape(224, 1024)
    base = {
        "ball0": np.ascontiguousarray(ballf[0:128]),
        "ball1": np.ascontiguousarray(ballf[128:224]),
        "blob": blob,
        "mrep2": mrep2,
        "pow2": np.ascontiguousarray(pow2.astype(np.float32)).astype(bf),
        "emb": np.ascontiguousarray(embedding, np.float32).reshape(-1),
        "head": np.ascontiguousarray(head_w, np.float32),
    }
    in_maps = []
    for c in range(NCORES):
        dslice = np.ascontiguousarray(
            d32[c * BC:(c + 1) * BC].T).astype(np.float32).astype(bf)
        in_maps.append({**base, "dataT": dslice})
    return in_maps


_CACHE = {}


def kernel(data, embedding, head_w, body_w, **kw):
    nc = _CACHE.get("nc")
    if nc is None:
        nc = build_program()
        _CACHE["nc"] = nc
    in_maps = make_in_maps(data, embedding, head_w, body_w)
    res = run_bass_kernel_spmd(nc, in_maps, core_ids=list(range(NCORES)))
    out = np.concatenate([res.results[c]["out"] for c in range(NCORES)], axis=0)
    return out.reshape(B, 2, 32)
